# revision 6
# baseline (speedup 1.0000x reference)
"""CodeAwareAttention Trainium2 kernel.

Shards batch (B=8) across 8 NeuronCores, one batch element per core.
Per core everything is computed in a "transposed" layout (feature dim on
partitions, tokens on the free axis) so that no large on-chip transposes of
the [S, S] attention matrices are needed:

  scoresT[k, q] = sum_d KT[d, k] * (Q[q, d]*gate'[q])  + cs'[q]   (aug row)
  probs_uT      = exp(scoresT) * exp(alignT)                       (softmax numerator)
  ctx_augT      = [V | 1]^T @ probs_uT          -> row 64 = softmax denominators
  probsT_out    = probs_uT * (1/den)[q]         (broadcast along partitions)
  ctx           = detranspose(ctx_augT) / den   (exact, per 128-token tile)

probs is written to HBM transposed ([H, S_k, S_q]); the host gather transposes
the view back to [B, H, S_q, S_k] (layout-only, no compute on host).

All matmuls run as float32r (full-rate fp32, ~2^-12 rounding on inputs).
"""

import numpy as np

import concourse.bass as bass
import concourse.tile as tile
from concourse import bacc, mybir
from concourse.masks import make_identity

F32 = mybir.dt.float32
F32R = mybir.dt.float32r
AF = mybir.ActivationFunctionType
ALU = mybir.AluOpType

B, S, D, H, DH = 8, 768, 768, 12, 64
NT = 6          # 128-row tiles per S/D
HALVES = ((0, 512), (512, 768))   # matmul moving-dim split; each fits one PSUM bank

_CACHE = {}


def _build():
    nc = bacc.Bacc("TRN2", target_bir_lowering=False, debug=False, num_devices=8)

    hs = nc.dram_tensor("hs", [S, D], F32, kind="ExternalInput").ap()
    ic = nc.dram_tensor("ic", [S, D], F32, kind="ExternalInput").ap()
    Wq = nc.dram_tensor("Wq", [D, D], F32, kind="ExternalInput").ap()
    Wk = nc.dram_tensor("Wk", [D, D], F32, kind="ExternalInput").ap()
    Wv = nc.dram_tensor("Wv", [D, D], F32, kind="ExternalInput").ap()
    Wa = nc.dram_tensor("Wa", [D, D], F32, kind="ExternalInput").ap()
    Wcs = nc.dram_tensor("Wcs", [D, H], F32, kind="ExternalInput").ap()
    Wg = nc.dram_tensor("Wg", [D, H], F32, kind="ExternalInput").ap()
    bq = nc.dram_tensor("bq", [D], F32, kind="ExternalInput").ap()
    bk = nc.dram_tensor("bk", [D], F32, kind="ExternalInput").ap()
    bv = nc.dram_tensor("bv", [D], F32, kind="ExternalInput").ap()
    ba = nc.dram_tensor("ba", [D], F32, kind="ExternalInput").ap()
    bcs = nc.dram_tensor("bcs", [H], F32, kind="ExternalInput").ap()
    bg = nc.dram_tensor("bg", [H], F32, kind="ExternalInput").ap()
    probsT = nc.dram_tensor("probsT", [H, S, S], F32, kind="ExternalOutput").ap()
    ctx_o = nc.dram_tensor("ctx", [S, D], F32, kind="ExternalOutput").ap()

    with tile.TileContext(nc) as tc:
        _emit(nc, tc, hs, ic, Wq, Wk, Wv, Wa, Wcs, Wg,
              bq, bk, bv, ba, bcs, bg, probsT, ctx_o)
    nc.compile()
    return nc


def _emit(nc, tc, hs, ic, Wq, Wk, Wv, Wa, Wcs, Wg,
          bq, bk, bv, ba, bcs, bg, probsT, ctx_o):
    from contextlib import ExitStack
    est = ExitStack()
    with est:
        cp = est.enter_context(tc.tile_pool(name="consts", bufs=1))
        pp = est.enter_context(tc.tile_pool(name="persist", bufs=1))

        ident = cp.tile([128, 128], F32, tag="ident")
        make_identity(nc, ident[:])
        ones_row = cp.tile([1, S], F32, tag="ones_row")
        nc.vector.memset(ones_row[:], 1.0)
        ones12 = cp.tile([128, H], F32, tag="ones12")
        nc.vector.memset(ones12[:], 1.0)

        bq_sb = cp.tile([128, NT], F32, tag="bq")
        nc.scalar.dma_start(bq_sb[:], bq.rearrange("(c p) -> p c", p=128))
        bk_sb = cp.tile([128, NT], F32, tag="bk")
        nc.scalar.dma_start(bk_sb[:], bk.rearrange("(c p) -> p c", p=128))
        ba_sb = cp.tile([128, NT], F32, tag="ba")
        nc.scalar.dma_start(ba_sb[:], ba.rearrange("(c p) -> p c", p=128))
        bcs_sb = cp.tile([H, 1], F32, tag="bcs")
        nc.scalar.dma_start(bcs_sb[:], bcs.rearrange("(h o) -> h o", o=1))
        bg_sb = cp.tile([H, 1], F32, tag="bg")
        nc.scalar.dma_start(bg_sb[:], bg.rearrange("(h o) -> h o", o=1))

        # persistent products of the preamble
        QA = [pp.tile([65, S], F32R, tag=f"QA{h}", name=f"QA{h}") for h in range(H)]
        KT = [pp.tile([65, S], F32R, tag=f"KT{h}", name=f"KT{h}") for h in range(H)]
        V = [pp.tile([128, H, 65], F32R, tag=f"V{t}", name=f"V{t}") for t in range(NT)]
        EA = [pp.tile([128, S], F32, tag=f"EA{j}", name=f"EA{j}") for j in range(NT)]

        with tc.tile_pool(name="phAB", bufs=1) as phAB:
            hsT = [phAB.tile([128, S], F32R, tag=f"hsT{j}", name=f"hsT{j}")
                   for j in range(NT)]
            icT = [phAB.tile([128, S], F32R, tag=f"icT{j}", name=f"icT{j}")
                   for j in range(NT)]

            # ---- Phase A: load hs/ic and transpose on the PE ----
            with tc.tile_pool(name="natp", bufs=2) as natp, \
                 tc.tile_pool(name="pstr", bufs=4, space="PSUM") as pstr:
                for src, dstT in ((hs, hsT), (ic, icT)):
                    for t2 in range(3):
                        nat = natp.tile([128, 2, S], F32, tag="nat")
                        nc.sync.dma_start(
                            nat[:], src[256 * t2:256 * t2 + 256, :]
                            .rearrange("(a p) d -> p a d", p=128))
                        for a in range(2):
                            i = 2 * t2 + a
                            for j in range(NT):
                                pt = pstr.tile([128, 128], F32, tag="tr")
                                nc.tensor.transpose(
                                    pt[:], nat[:, a, 128 * j:128 * j + 128],
                                    ident[:])
                                dst = dstT[j][:, 128 * i:128 * i + 128]
                                if j % 2 == 0:
                                    nc.scalar.copy(dst, pt[:])
                                else:
                                    nc.vector.tensor_copy(dst, pt[:])

            # ---- Phase B: projections ----
            with tc.tile_pool(name="wp", bufs=7) as wp, \
                 tc.tile_pool(name="phB", bufs=1) as phB, \
                 tc.tile_pool(name="stg", bufs=2) as stg, \
                 tc.tile_pool(name="tmpp", bufs=2) as tmpp, \
                 tc.tile_pool(name="gbp", bufs=2) as gbp, \
                 tc.tile_pool(name="psb", bufs=2, space="PSUM") as psb:

                def load_w(Wdram, label):
                    tiles = []
                    for c in range(NT):
                        t = wp.tile([128, S], F32R, tag="w", name=f"{label}{c}")
                        nc.sync.dma_start(
                            t[:], Wdram[128 * c:128 * c + 128, :].bitcast(F32R))
                        tiles.append(t)
                    return tiles

                # cs / gate projections ([12, S] each, partitions 0-11)
                wcsg = phB.tile([128, NT, 2 * H], F32R, tag="wcsg")
                nc.scalar.dma_start(
                    wcsg[:, :, 0:H],
                    Wcs.rearrange("(c p) h -> p c h", p=128).bitcast(F32R))
                nc.scalar.dma_start(
                    wcsg[:, :, H:2 * H],
                    Wg.rearrange("(c p) h -> p c h", p=128).bitcast(F32R))
                ps_cs = psb.tile([H, S], F32, tag="csg")
                ps_g = psb.tile([H, S], F32, tag="csg")
                for c in range(NT):
                    for lo, hi in HALVES:
                        nc.tensor.matmul(ps_cs[:, lo:hi], wcsg[:, c, 0:H],
                                         hsT[c][:, lo:hi],
                                         start=(c == 0), stop=(c == NT - 1))
                        nc.tensor.matmul(ps_g[:, lo:hi], wcsg[:, c, H:2 * H],
                                         hsT[c][:, lo:hi],
                                         start=(c == 0), stop=(c == NT - 1))
                cs_sb = phB.tile([H, S], F32, tag="cs")
                nc.scalar.add(cs_sb[:], ps_cs[:], bcs_sb[:])
                gate_sb = phB.tile([H, S], F32, tag="gate")
                nc.scalar.activation(gate_sb[:], ps_g[:], AF.Sigmoid,
                                     bias=bg_sb[:], scale=1.0)
                cprime = phB.tile([H, S], F32, tag="cp")
                nc.vector.tensor_mul(cprime[:], cs_sb[:], gate_sb[:])
                gate8 = phB.tile([H, S], F32, tag="g8")
                nc.vector.tensor_scalar_mul(gate8[:], gate_sb[:], 0.125)

                # Q projection, fused with gate prescale -> QA head tiles
                wq_sb = load_w(Wq, "wq")
                for m in range(NT):
                    pj = psb.tile([128, S], F32, tag="proj")
                    for c in range(NT):
                        for lo, hi in HALVES:
                            nc.tensor.matmul(pj[:, lo:hi],
                                             wq_sb[c][:, 128 * m:128 * m + 128],
                                             hsT[c][:, lo:hi],
                                             start=(c == 0), stop=(c == NT - 1))
                    he, ho = 2 * m, 2 * m + 1
                    stge = stg.tile([1, S], F32, tag="stg")
                    nc.scalar.dma_start(stge[:], gate8[he:he + 1, :])
                    gbe = gbp.tile([128, S], F32, tag="gb")
                    nc.gpsimd.partition_broadcast(gbe[:], stge[:])
                    stgo = stg.tile([1, S], F32, tag="stg")
                    nc.scalar.dma_start(stgo[:], gate8[ho:ho + 1, :])
                    gbo = gbp.tile([128, S], F32, tag="gb")
                    nc.gpsimd.partition_broadcast(gbo[:], stgo[:])
                    nc.vector.scalar_tensor_tensor(
                        QA[he][0:64, :], pj[0:64, :], bq_sb[0:64, m:m + 1],
                        gbe[0:64, :], op0=ALU.add, op1=ALU.mult)
                    qtmp = tmpp.tile([128, S], F32R, tag="qtmp")
                    nc.vector.scalar_tensor_tensor(
                        qtmp[64:128, :], pj[64:128, :], bq_sb[64:128, m:m + 1],
                        gbo[64:128, :], op0=ALU.add, op1=ALU.mult)
                    nc.scalar.dma_start(QA[ho][0:64, :], qtmp[64:128, :])
                    nc.scalar.dma_start(QA[he][64:65, :],
                                        cprime[he:he + 1, :].bitcast(F32R))
                    nc.scalar.dma_start(QA[ho][64:65, :],
                                        cprime[ho:ho + 1, :].bitcast(F32R))

                # K projection -> KT head tiles (row 64 = ones)
                wk_sb = load_w(Wk, "wk")
                for m in range(NT):
                    pj = psb.tile([128, S], F32, tag="proj")
                    for c in range(NT):
                        for lo, hi in HALVES:
                            nc.tensor.matmul(pj[:, lo:hi],
                                             wk_sb[c][:, 128 * m:128 * m + 128],
                                             hsT[c][:, lo:hi],
                                             start=(c == 0), stop=(c == NT - 1))
                    he, ho = 2 * m, 2 * m + 1
                    nc.scalar.add(KT[he][0:64, :], pj[0:64, :], bk_sb[0:64, m:m + 1])
                    ktmp = tmpp.tile([128, S], F32R, tag="qtmp")
                    nc.scalar.add(ktmp[64:128, :], pj[64:128, :],
                                  bk_sb[64:128, m:m + 1])
                    nc.scalar.dma_start(KT[ho][0:64, :], ktmp[64:128, :])
                    nc.scalar.dma_start(KT[he][64:65, :], ones_row[:].bitcast(F32R))
                    nc.scalar.dma_start(KT[ho][64:65, :], ones_row[:].bitcast(F32R))

                # V projection (natural layout, per-head 65-wide with ones col)
                wv_sb = load_w(Wv, "wv")
                stbv = stg.tile([1, S], F32, tag="stg")
                nc.scalar.dma_start(stbv[:], bv.rearrange("(o d) -> o d", o=1))
                bv_b = phB.tile([128, S], F32, tag="bvb")
                nc.gpsimd.partition_broadcast(bv_b[:], stbv[:])
                for t in range(NT):
                    pj = psb.tile([128, S], F32, tag="proj")
                    for c in range(NT):
                        for lo, hi in HALVES:
                            nc.tensor.matmul(pj[:, lo:hi],
                                             hsT[c][:, 128 * t:128 * t + 128],
                                             wv_sb[c][:, lo:hi],
                                             start=(c == 0), stop=(c == NT - 1))
                    nc.vector.tensor_add(
                        V[t][:, :, 0:64],
                        pj[:].rearrange("p (h d) -> p h d", h=H),
                        bv_b[:].rearrange("p (h d) -> p h d", h=H))
                    nc.scalar.dma_start(
                        V[t][:, :, 64:65],
                        ones12[:].rearrange("p (h o) -> p h o", o=1).bitcast(F32R))

                # align projection -> exp(alignT + ba) directly
                wa_sb = load_w(Wa, "wa")
                for kt in range(NT):
                    pj = psb.tile([128, S], F32, tag="proj")
                    for c in range(NT):
                        for lo, hi in HALVES:
                            nc.tensor.matmul(pj[:, lo:hi],
                                             wa_sb[c][:, 128 * kt:128 * kt + 128],
                                             icT[c][:, lo:hi],
                                             start=(c == 0), stop=(c == NT - 1))
                    nc.scalar.activation(EA[kt][:], pj[:], AF.Exp,
                                         bias=ba_sb[:, kt:kt + 1], scale=1.0)

        # ---- Phase C: attention per head ----
        with tc.tile_pool(name="ctxp", bufs=1) as ctxp, \
             tc.tile_pool(name="e1p", bufs=3) as e1p, \
             tc.tile_pool(name="pup", bufs=8) as pup, \
             tc.tile_pool(name="pop", bufs=2) as pop, \
             tc.tile_pool(name="catp", bufs=2) as catp, \
             tc.tile_pool(name="invbp", bufs=2) as invbp, \
             tc.tile_pool(name="stgC", bufs=2) as stgC, \
             tc.tile_pool(name="psS", bufs=2, space="PSUM") as psS, \
             tc.tile_pool(name="psC", bufs=1, space="PSUM") as psC, \
             tc.tile_pool(name="psT", bufs=2, space="PSUM") as psT:

            ctx_sb = [ctxp.tile([128, 2, S], F32, tag=f"ctx{t2}", name=f"ctxsb{t2}")
                      for t2 in range(3)]

            for h in range(H):
                pu_tiles = []
                for kt in range(NT):
                    ps_s = psS.tile([128, S], F32, tag="sc")
                    for lo, hi in HALVES:
                        nc.tensor.matmul(ps_s[:, lo:hi],
                                         KT[h][:, 128 * kt:128 * kt + 128],
                                         QA[h][:, lo:hi], start=True, stop=True)
                    e1 = e1p.tile([128, S], F32, tag="e1")
                    nc.scalar.activation(e1[:], ps_s[:], AF.Exp)
                    pu = pup.tile([128, S], F32R, tag="pu")
                    eng = nc.vector if kt % 2 == 0 else nc.gpsimd
                    eng.tensor_mul(pu[:], e1[:], EA[kt][:])
                    pu_tiles.append(pu)

                ps_c = psC.tile([65, S], F32, tag="cx")
                for kt in range(NT):
                    for lo, hi in HALVES:
                        nc.tensor.matmul(ps_c[:, lo:hi], V[kt][:, h, :],
                                         pu_tiles[kt][:, lo:hi],
                                         start=(kt == 0), stop=(kt == NT - 1))
                cat = catp.tile([65, S], F32, tag="cat")
                nc.scalar.copy(cat[:], ps_c[:])
                den = stgC.tile([1, S], F32, tag="den")
                nc.scalar.dma_start(den[:], cat[64:65, :])
                inv = stgC.tile([1, S], F32, tag="inv")
                nc.vector.reciprocal_approx_fast(out=inv[:], in_=den[:])
                invb = invbp.tile([128, S], F32, tag="invb")
                nc.gpsimd.partition_broadcast(invb[:], inv[:])

                for g in range(2):
                    po = pop.tile([128, 3, S], F32, tag="po")
                    for j in range(3):
                        kt = 3 * g + j
                        eng = nc.gpsimd if kt % 2 == 0 else nc.vector
                        eng.tensor_mul(po[:, j, :],
                                       pu_tiles[kt][:].bitcast(F32), invb[:])
                    nc.sync.dma_start(
                        probsT[h, 384 * g:384 * g + 384, :]
                        .rearrange("(j p) d -> p j d", p=128),
                        po[:])

                # ctx: detranspose ctx_augT, divide by den, accumulate to ctx_sb
                pt6 = psT.tile([128, NT, 80], F32, tag="dt")
                for qt in range(NT):
                    nc.tensor.transpose(pt6[:, qt, 0:65],
                                        cat[:, 128 * qt:128 * qt + 128],
                                        ident[0:65, 0:65])
                inv6 = stgC.tile([128, NT], F32, tag="i6")
                nc.vector.reciprocal(
                    inv6[:], pt6[:, :, 64:65].rearrange("p c o -> p (c o)"))
                for qt in range(NT):
                    nc.vector.tensor_scalar_mul(
                        ctx_sb[qt // 2][:, qt % 2, 64 * h:64 * h + 64],
                        pt6[:, qt, 0:64], inv6[:, qt:qt + 1])

            for t2 in range(3):
                nc.sync.dma_start(
                    ctx_o[256 * t2:256 * t2 + 256, :]
                    .rearrange("(a p) d -> p a d", p=128),
                    ctx_sb[t2][:])


def _make_exec(nc):
    """Build a cached jitted SPMD executor (mirrors bass2jax.run_bass_via_pjrt)."""
    import jax
    from jax.sharding import Mesh, PartitionSpec
    from jax.experimental.shard_map import shard_map
    from concourse import bass2jax as b2j
    from concourse import mybir as _mybir

    b2j.install_neuronx_cc_hook()
    partition_name = (nc.partition_id_tensor.name
                      if nc.partition_id_tensor else None)

    assert nc.dbg_addr is None, "build with debug=False"
    in_names, out_names, out_avals = [], [], []
    for alloc in nc.m.functions[0].allocations:
        if not isinstance(alloc, _mybir.MemoryLocationSet):
            continue
        name = alloc.memorylocations[0].name
        if alloc.kind == "ExternalInput":
            if name != partition_name:
                in_names.append(name)
        elif alloc.kind == "ExternalOutput":
            out_names.append(name)
            out_avals.append(jax.core.ShapedArray(
                tuple(alloc.tensor_shape), _mybir.dt.np(alloc.dtype)))

    n_params = len(in_names)
    n_outs = len(out_names)
    all_in_names = list(in_names) + list(out_names)
    if partition_name is not None:
        all_in_names.append(partition_name)
    donate = tuple(range(n_params, n_params + n_outs))

    def _body(*args):
        operands = list(args)
        if partition_name is not None:
            operands.append(b2j.partition_id_tensor())
        outs = b2j._bass_exec_p.bind(
            *operands,
            out_avals=tuple(jax.core.ShapedArray(a.shape, a.dtype)
                            for a in out_avals),
            in_names=tuple(all_in_names),
            out_names=tuple(out_names),
            lowering_input_output_aliases=(),
            sim_require_finite=True,
            sim_require_nnan=True,
            nc=nc,
        )
        return tuple(outs)

    n_cores = 8
    devices = jax.devices()[:n_cores]
    mesh = Mesh(np.asarray(devices), ("core",))
    in_specs = (PartitionSpec("core"),) * (n_params + n_outs)
    out_specs = (PartitionSpec("core"),) * n_outs
    sharded = jax.jit(
        shard_map(_body, mesh=mesh, in_specs=in_specs, out_specs=out_specs,
                  check_rep=False),
        donate_argnums=donate, keep_unused=True)

    def put_inputs(in_maps):
        return [np.concatenate([np.asarray(in_maps[c][nm])[None]
                                if np.asarray(in_maps[c][nm]).ndim == 0
                                else np.asarray(in_maps[c][nm])
                                for c in range(n_cores)], axis=0)
                for nm in in_names]

    def execute_raw(device_inputs, zeros):
        return sharded(*device_inputs, *zeros)

    def execute(in_maps):
        device_inputs = put_inputs(in_maps)
        zeros = [np.zeros((n_cores * a.shape[0], *a.shape[1:]), a.dtype)
                 for a in out_avals]
        out_arrs = sharded(*device_inputs, *zeros)
        return [
            {name: np.asarray(out_arrs[i]).reshape(n_cores, *out_avals[i].shape)[c]
             for i, name in enumerate(out_names)}
            for c in range(n_cores)
        ]

    execute.put_inputs = put_inputs
    execute.raw = execute_raw
    execute.out_avals = out_avals
    execute.out_names = out_names
    execute.n_cores = n_cores
    return execute


def _get_runner():
    if "exec" not in _CACHE:
        nc = _build()
        _CACHE["exec"] = _make_exec(nc)
    return _CACHE["exec"]


def _shard(inputs):
    hs = np.asarray(inputs["hidden_states"], np.float32)
    ic = np.asarray(inputs["instruction_context"], np.float32)
    shared = {n: np.asarray(inputs[n], np.float32) for n in (
        "Wq", "Wk", "Wv", "Wa", "Wcs", "Wg",
        "bq", "bk", "bv", "ba", "bcs", "bg")}
    return [{"hs": hs[b], "ic": ic[b], **shared} for b in range(B)]


def kernel(**inputs):
    runner = _get_runner()
    results = runner(_shard(inputs))
    ctx = np.stack([r["ctx"] for r in results])             # [B, S, D]
    probsT = np.stack([r["probsT"] for r in results])       # [B, H, S_k, S_q]
    probs = probsT.transpose(0, 1, 3, 2)                    # [B, H, S_q, S_k]
    return ctx, probs


# revision 8
# speedup vs baseline: 73.1889x; 73.1889x over previous
"""CodeAwareAttention Trainium2 kernel.

Shards batch (B=8) across 8 NeuronCores, one batch element per core.
Per core everything is computed in a "transposed" layout (feature dim on
partitions, tokens on the free axis) so that no large on-chip transposes of
the [S, S] attention matrices are needed:

  scoresT[k, q] = sum_d KT[d, k] * (Q[q, d]*gate'[q])  + cs'[q]   (aug row)
  probs_uT      = exp(scoresT) * exp(alignT)                       (softmax numerator)
  ctx_augT      = [V | 1]^T @ probs_uT          -> row 64 = softmax denominators
  probsT_out    = probs_uT * (1/den)[q]         (broadcast along partitions)
  ctx           = detranspose(ctx_augT) / den   (exact, per 128-token tile)

probs is written to HBM transposed ([H, S_k, S_q]); the host gather transposes
the view back to [B, H, S_q, S_k] (layout-only, no compute on host).

All matmuls run as float32r (full-rate fp32, ~2^-12 rounding on inputs).
"""

import numpy as np

import concourse.bass as bass
import concourse.tile as tile
from concourse import bacc, mybir
from concourse.masks import make_identity

F32 = mybir.dt.float32
F32R = mybir.dt.float32r
AF = mybir.ActivationFunctionType
ALU = mybir.AluOpType

B, S, D, H, DH = 8, 768, 768, 12, 64
NT = 6          # 128-row tiles per S/D
HALVES = ((0, 512), (512, 768))   # matmul moving-dim split; each fits one PSUM bank

_CACHE = {}


def _build():
    nc = bacc.Bacc("TRN2", target_bir_lowering=False, debug=False, num_devices=8)

    hs = nc.dram_tensor("hs", [S, D], F32, kind="ExternalInput").ap()
    ic = nc.dram_tensor("ic", [S, D], F32, kind="ExternalInput").ap()
    Wq = nc.dram_tensor("Wq", [D, D], F32, kind="ExternalInput").ap()
    Wk = nc.dram_tensor("Wk", [D, D], F32, kind="ExternalInput").ap()
    Wv = nc.dram_tensor("Wv", [D, D], F32, kind="ExternalInput").ap()
    Wa = nc.dram_tensor("Wa", [D, D], F32, kind="ExternalInput").ap()
    Wcs = nc.dram_tensor("Wcs", [D, H], F32, kind="ExternalInput").ap()
    Wg = nc.dram_tensor("Wg", [D, H], F32, kind="ExternalInput").ap()
    bq = nc.dram_tensor("bq", [D], F32, kind="ExternalInput").ap()
    bk = nc.dram_tensor("bk", [D], F32, kind="ExternalInput").ap()
    bv = nc.dram_tensor("bv", [D], F32, kind="ExternalInput").ap()
    ba = nc.dram_tensor("ba", [D], F32, kind="ExternalInput").ap()
    bcs = nc.dram_tensor("bcs", [H], F32, kind="ExternalInput").ap()
    bg = nc.dram_tensor("bg", [H], F32, kind="ExternalInput").ap()
    probsT = nc.dram_tensor("probsT", [H, S, S], F32, kind="ExternalOutput").ap()
    ctx_o = nc.dram_tensor("ctx", [S, D], F32, kind="ExternalOutput").ap()

    with tile.TileContext(nc) as tc:
        _emit(nc, tc, hs, ic, Wq, Wk, Wv, Wa, Wcs, Wg,
              bq, bk, bv, ba, bcs, bg, probsT, ctx_o)
    nc.compile()
    return nc


def _emit(nc, tc, hs, ic, Wq, Wk, Wv, Wa, Wcs, Wg,
          bq, bk, bv, ba, bcs, bg, probsT, ctx_o):
    from contextlib import ExitStack
    est = ExitStack()
    with est:
        cp = est.enter_context(tc.tile_pool(name="consts", bufs=1))
        pp = est.enter_context(tc.tile_pool(name="persist", bufs=1))

        ident = cp.tile([128, 128], F32, tag="ident")
        make_identity(nc, ident[:])
        ones_row = cp.tile([1, S], F32, tag="ones_row")
        nc.vector.memset(ones_row[:], 1.0)
        ones12 = cp.tile([128, H], F32, tag="ones12")
        nc.vector.memset(ones12[:], 1.0)

        bq_sb = cp.tile([128, NT], F32, tag="bq")
        nc.scalar.dma_start(bq_sb[:], bq.rearrange("(c p) -> p c", p=128))
        bk_sb = cp.tile([128, NT], F32, tag="bk")
        nc.scalar.dma_start(bk_sb[:], bk.rearrange("(c p) -> p c", p=128))
        ba_sb = cp.tile([128, NT], F32, tag="ba")
        nc.scalar.dma_start(ba_sb[:], ba.rearrange("(c p) -> p c", p=128))
        bcs_sb = cp.tile([H, 1], F32, tag="bcs")
        nc.scalar.dma_start(bcs_sb[:], bcs.rearrange("(h o) -> h o", o=1))
        bg_sb = cp.tile([H, 1], F32, tag="bg")
        nc.scalar.dma_start(bg_sb[:], bg.rearrange("(h o) -> h o", o=1))

        # persistent products of the preamble
        QA = [pp.tile([65, S], F32R, tag=f"QA{h}", name=f"QA{h}") for h in range(H)]
        KT = [pp.tile([65, S], F32R, tag=f"KT{h}", name=f"KT{h}") for h in range(H)]
        V = [pp.tile([128, H, 65], F32R, tag=f"V{t}", name=f"V{t}") for t in range(NT)]
        EA = [pp.tile([128, S], F32, tag=f"EA{j}", name=f"EA{j}") for j in range(NT)]

        with tc.tile_pool(name="phAB", bufs=1) as phAB:
            hsT = [phAB.tile([128, S], F32R, tag=f"hsT{j}", name=f"hsT{j}")
                   for j in range(NT)]
            icT = [phAB.tile([128, S], F32R, tag=f"icT{j}", name=f"icT{j}")
                   for j in range(NT)]

            # ---- Phase A: load hs/ic and transpose on the PE ----
            with tc.tile_pool(name="natp", bufs=2) as natp, \
                 tc.tile_pool(name="pstr", bufs=4, space="PSUM") as pstr:
                for src, dstT in ((hs, hsT), (ic, icT)):
                    for t2 in range(3):
                        nat = natp.tile([128, 2, S], F32, tag="nat")
                        nc.sync.dma_start(
                            nat[:], src[256 * t2:256 * t2 + 256, :]
                            .rearrange("(a p) d -> p a d", p=128))
                        for a in range(2):
                            i = 2 * t2 + a
                            for j in range(NT):
                                pt = pstr.tile([128, 128], F32, tag="tr")
                                nc.tensor.transpose(
                                    pt[:], nat[:, a, 128 * j:128 * j + 128],
                                    ident[:])
                                dst = dstT[j][:, 128 * i:128 * i + 128]
                                if j % 2 == 0:
                                    nc.scalar.copy(dst, pt[:])
                                else:
                                    nc.vector.tensor_copy(dst, pt[:])

            # ---- Phase B: projections ----
            with tc.tile_pool(name="wp", bufs=7) as wp, \
                 tc.tile_pool(name="phB", bufs=1) as phB, \
                 tc.tile_pool(name="stg", bufs=2) as stg, \
                 tc.tile_pool(name="tmpp", bufs=2) as tmpp, \
                 tc.tile_pool(name="gbp", bufs=2) as gbp, \
                 tc.tile_pool(name="psb", bufs=2, space="PSUM") as psb:

                def load_w(Wdram, label):
                    tiles = []
                    for c in range(NT):
                        t = wp.tile([128, S], F32R, tag="w", name=f"{label}{c}")
                        nc.sync.dma_start(
                            t[:], Wdram[128 * c:128 * c + 128, :].bitcast(F32R))
                        tiles.append(t)
                    return tiles

                # cs / gate projections ([12, S] each, partitions 0-11)
                wcsg = phB.tile([128, NT, 2 * H], F32R, tag="wcsg")
                nc.scalar.dma_start(
                    wcsg[:, :, 0:H],
                    Wcs.rearrange("(c p) h -> p c h", p=128).bitcast(F32R))
                nc.scalar.dma_start(
                    wcsg[:, :, H:2 * H],
                    Wg.rearrange("(c p) h -> p c h", p=128).bitcast(F32R))
                ps_cs = psb.tile([H, S], F32, tag="csg")
                ps_g = psb.tile([H, S], F32, tag="csg")
                for c in range(NT):
                    for lo, hi in HALVES:
                        nc.tensor.matmul(ps_cs[:, lo:hi], wcsg[:, c, 0:H],
                                         hsT[c][:, lo:hi],
                                         start=(c == 0), stop=(c == NT - 1))
                        nc.tensor.matmul(ps_g[:, lo:hi], wcsg[:, c, H:2 * H],
                                         hsT[c][:, lo:hi],
                                         start=(c == 0), stop=(c == NT - 1))
                cs_sb = phB.tile([H, S], F32, tag="cs")
                nc.scalar.add(cs_sb[:], ps_cs[:], bcs_sb[:])
                gate_sb = phB.tile([H, S], F32, tag="gate")
                nc.scalar.activation(gate_sb[:], ps_g[:], AF.Sigmoid,
                                     bias=bg_sb[:], scale=1.0)
                cprime = phB.tile([H, S], F32, tag="cp")
                nc.vector.tensor_mul(cprime[:], cs_sb[:], gate_sb[:])
                gate8 = phB.tile([H, S], F32, tag="g8")
                nc.vector.tensor_scalar_mul(gate8[:], gate_sb[:], 0.125)

                # Q projection, fused with gate prescale -> QA head tiles
                wq_sb = load_w(Wq, "wq")
                for m in range(NT):
                    pj = psb.tile([128, S], F32, tag="proj")
                    for c in range(NT):
                        for lo, hi in HALVES:
                            nc.tensor.matmul(pj[:, lo:hi],
                                             wq_sb[c][:, 128 * m:128 * m + 128],
                                             hsT[c][:, lo:hi],
                                             start=(c == 0), stop=(c == NT - 1))
                    he, ho = 2 * m, 2 * m + 1
                    stge = stg.tile([1, S], F32, tag="stg")
                    nc.scalar.dma_start(stge[:], gate8[he:he + 1, :])
                    gbe = gbp.tile([128, S], F32, tag="gb")
                    nc.gpsimd.partition_broadcast(gbe[:], stge[:])
                    stgo = stg.tile([1, S], F32, tag="stg")
                    nc.scalar.dma_start(stgo[:], gate8[ho:ho + 1, :])
                    gbo = gbp.tile([128, S], F32, tag="gb")
                    nc.gpsimd.partition_broadcast(gbo[:], stgo[:])
                    nc.vector.scalar_tensor_tensor(
                        QA[he][0:64, :], pj[0:64, :], bq_sb[0:64, m:m + 1],
                        gbe[0:64, :], op0=ALU.add, op1=ALU.mult)
                    qtmp = tmpp.tile([128, S], F32R, tag="qtmp")
                    nc.vector.scalar_tensor_tensor(
                        qtmp[64:128, :], pj[64:128, :], bq_sb[64:128, m:m + 1],
                        gbo[64:128, :], op0=ALU.add, op1=ALU.mult)
                    nc.scalar.dma_start(QA[ho][0:64, :], qtmp[64:128, :])
                    nc.scalar.dma_start(QA[he][64:65, :],
                                        cprime[he:he + 1, :].bitcast(F32R))
                    nc.scalar.dma_start(QA[ho][64:65, :],
                                        cprime[ho:ho + 1, :].bitcast(F32R))

                # K projection -> KT head tiles (row 64 = ones)
                wk_sb = load_w(Wk, "wk")
                for m in range(NT):
                    pj = psb.tile([128, S], F32, tag="proj")
                    for c in range(NT):
                        for lo, hi in HALVES:
                            nc.tensor.matmul(pj[:, lo:hi],
                                             wk_sb[c][:, 128 * m:128 * m + 128],
                                             hsT[c][:, lo:hi],
                                             start=(c == 0), stop=(c == NT - 1))
                    he, ho = 2 * m, 2 * m + 1
                    nc.scalar.add(KT[he][0:64, :], pj[0:64, :], bk_sb[0:64, m:m + 1])
                    ktmp = tmpp.tile([128, S], F32R, tag="qtmp")
                    nc.scalar.add(ktmp[64:128, :], pj[64:128, :],
                                  bk_sb[64:128, m:m + 1])
                    nc.scalar.dma_start(KT[ho][0:64, :], ktmp[64:128, :])
                    nc.scalar.dma_start(KT[he][64:65, :], ones_row[:].bitcast(F32R))
                    nc.scalar.dma_start(KT[ho][64:65, :], ones_row[:].bitcast(F32R))

                # V projection (natural layout, per-head 65-wide with ones col)
                wv_sb = load_w(Wv, "wv")
                stbv = stg.tile([1, S], F32, tag="stg")
                nc.scalar.dma_start(stbv[:], bv.rearrange("(o d) -> o d", o=1))
                bv_b = phB.tile([128, S], F32, tag="bvb")
                nc.gpsimd.partition_broadcast(bv_b[:], stbv[:])
                for t in range(NT):
                    pj = psb.tile([128, S], F32, tag="proj")
                    for c in range(NT):
                        for lo, hi in HALVES:
                            nc.tensor.matmul(pj[:, lo:hi],
                                             hsT[c][:, 128 * t:128 * t + 128],
                                             wv_sb[c][:, lo:hi],
                                             start=(c == 0), stop=(c == NT - 1))
                    nc.vector.tensor_add(
                        V[t][:, :, 0:64],
                        pj[:].rearrange("p (h d) -> p h d", h=H),
                        bv_b[:].rearrange("p (h d) -> p h d", h=H))
                    nc.scalar.dma_start(
                        V[t][:, :, 64:65],
                        ones12[:].rearrange("p (h o) -> p h o", o=1).bitcast(F32R))

                # align projection -> exp(alignT + ba) directly
                wa_sb = load_w(Wa, "wa")
                for kt in range(NT):
                    pj = psb.tile([128, S], F32, tag="proj")
                    for c in range(NT):
                        for lo, hi in HALVES:
                            nc.tensor.matmul(pj[:, lo:hi],
                                             wa_sb[c][:, 128 * kt:128 * kt + 128],
                                             icT[c][:, lo:hi],
                                             start=(c == 0), stop=(c == NT - 1))
                    nc.scalar.activation(EA[kt][:], pj[:], AF.Exp,
                                         bias=ba_sb[:, kt:kt + 1], scale=1.0)

        # ---- Phase C: attention per head ----
        with tc.tile_pool(name="ctxp", bufs=1) as ctxp, \
             tc.tile_pool(name="e1p", bufs=3) as e1p, \
             tc.tile_pool(name="pup", bufs=8) as pup, \
             tc.tile_pool(name="pop", bufs=2) as pop, \
             tc.tile_pool(name="catp", bufs=2) as catp, \
             tc.tile_pool(name="invbp", bufs=2) as invbp, \
             tc.tile_pool(name="stgC", bufs=2) as stgC, \
             tc.tile_pool(name="psS", bufs=2, space="PSUM") as psS, \
             tc.tile_pool(name="psC", bufs=1, space="PSUM") as psC, \
             tc.tile_pool(name="psT", bufs=2, space="PSUM") as psT:

            ctx_sb = [ctxp.tile([128, 2, S], F32, tag=f"ctx{t2}", name=f"ctxsb{t2}")
                      for t2 in range(3)]

            for h in range(H):
                pu_tiles = []
                for kt in range(NT):
                    ps_s = psS.tile([128, S], F32, tag="sc")
                    for lo, hi in HALVES:
                        nc.tensor.matmul(ps_s[:, lo:hi],
                                         KT[h][:, 128 * kt:128 * kt + 128],
                                         QA[h][:, lo:hi], start=True, stop=True)
                    e1 = e1p.tile([128, S], F32, tag="e1")
                    nc.scalar.activation(e1[:], ps_s[:], AF.Exp)
                    pu = pup.tile([128, S], F32R, tag="pu")
                    eng = nc.vector if kt % 2 == 0 else nc.gpsimd
                    eng.tensor_mul(pu[:], e1[:], EA[kt][:])
                    pu_tiles.append(pu)

                ps_c = psC.tile([65, S], F32, tag="cx")
                for kt in range(NT):
                    for lo, hi in HALVES:
                        nc.tensor.matmul(ps_c[:, lo:hi], V[kt][:, h, :],
                                         pu_tiles[kt][:, lo:hi],
                                         start=(kt == 0), stop=(kt == NT - 1))
                cat = catp.tile([65, S], F32, tag="cat")
                nc.scalar.copy(cat[:], ps_c[:])
                den = stgC.tile([1, S], F32, tag="den")
                nc.scalar.dma_start(den[:], cat[64:65, :])
                inv = stgC.tile([1, S], F32, tag="inv")
                nc.vector.reciprocal_approx_fast(out=inv[:], in_=den[:])
                invb = invbp.tile([128, S], F32, tag="invb")
                nc.gpsimd.partition_broadcast(invb[:], inv[:])

                for g in range(2):
                    po = pop.tile([128, 3, S], F32, tag="po")
                    for j in range(3):
                        kt = 3 * g + j
                        eng = nc.gpsimd if kt % 2 == 0 else nc.vector
                        eng.tensor_mul(po[:, j, :],
                                       pu_tiles[kt][:].bitcast(F32), invb[:])
                    nc.sync.dma_start(
                        probsT[h, 384 * g:384 * g + 384, :]
                        .rearrange("(j p) d -> p j d", p=128),
                        po[:])

                # ctx: detranspose ctx_augT, divide by den, accumulate to ctx_sb
                pt6 = psT.tile([128, NT, 80], F32, tag="dt")
                for qt in range(NT):
                    nc.tensor.transpose(pt6[:, qt, 0:65],
                                        cat[:, 128 * qt:128 * qt + 128],
                                        ident[0:65, 0:65])
                inv6 = stgC.tile([128, NT], F32, tag="i6")
                nc.vector.reciprocal(
                    inv6[:], pt6[:, :, 64:65].rearrange("p c o -> p (c o)"))
                for qt in range(NT):
                    nc.vector.tensor_scalar_mul(
                        ctx_sb[qt // 2][:, qt % 2, 64 * h:64 * h + 64],
                        pt6[:, qt, 0:64], inv6[:, qt:qt + 1])

            for t2 in range(3):
                nc.sync.dma_start(
                    ctx_o[256 * t2:256 * t2 + 256, :]
                    .rearrange("(a p) d -> p a d", p=128),
                    ctx_sb[t2][:])


def _make_exec(nc):
    """Build a cached jitted SPMD executor (mirrors bass2jax.run_bass_via_pjrt)."""
    import jax
    from jax.sharding import Mesh, PartitionSpec
    from jax.experimental.shard_map import shard_map
    from concourse import bass2jax as b2j
    from concourse import mybir as _mybir

    b2j.install_neuronx_cc_hook()
    partition_name = (nc.partition_id_tensor.name
                      if nc.partition_id_tensor else None)

    assert nc.dbg_addr is None, "build with debug=False"
    in_names, out_names, out_avals = [], [], []
    for alloc in nc.m.functions[0].allocations:
        if not isinstance(alloc, _mybir.MemoryLocationSet):
            continue
        name = alloc.memorylocations[0].name
        if alloc.kind == "ExternalInput":
            if name != partition_name:
                in_names.append(name)
        elif alloc.kind == "ExternalOutput":
            out_names.append(name)
            out_avals.append(jax.core.ShapedArray(
                tuple(alloc.tensor_shape), _mybir.dt.np(alloc.dtype)))

    n_params = len(in_names)
    n_outs = len(out_names)
    all_in_names = list(in_names) + list(out_names)
    if partition_name is not None:
        all_in_names.append(partition_name)
    donate = tuple(range(n_params, n_params + n_outs))

    def _body(*args):
        operands = list(args)
        if partition_name is not None:
            operands.append(b2j.partition_id_tensor())
        outs = b2j._bass_exec_p.bind(
            *operands,
            out_avals=tuple(jax.core.ShapedArray(a.shape, a.dtype)
                            for a in out_avals),
            in_names=tuple(all_in_names),
            out_names=tuple(out_names),
            lowering_input_output_aliases=(),
            sim_require_finite=True,
            sim_require_nnan=True,
            nc=nc,
        )
        return tuple(outs)

    n_cores = 8
    devices = jax.devices()[:n_cores]
    mesh = Mesh(np.asarray(devices), ("core",))
    in_specs = (PartitionSpec("core"),) * (n_params + n_outs)
    out_specs = (PartitionSpec("core"),) * n_outs
    sharded = jax.jit(
        shard_map(_body, mesh=mesh, in_specs=in_specs, out_specs=out_specs,
                  check_rep=False),
        donate_argnums=donate, keep_unused=True)

    def put_inputs(in_maps):
        return [np.concatenate([np.asarray(in_maps[c][nm])[None]
                                if np.asarray(in_maps[c][nm]).ndim == 0
                                else np.asarray(in_maps[c][nm])
                                for c in range(n_cores)], axis=0)
                for nm in in_names]

    def execute_raw(device_inputs, zeros):
        return sharded(*device_inputs, *zeros)

    def execute(in_maps):
        device_inputs = put_inputs(in_maps)
        zeros = [np.zeros((n_cores * a.shape[0], *a.shape[1:]), a.dtype)
                 for a in out_avals]
        out_arrs = sharded(*device_inputs, *zeros)
        return [
            {name: np.asarray(out_arrs[i]).reshape(n_cores, *out_avals[i].shape)[c]
             for i, name in enumerate(out_names)}
            for c in range(n_cores)
        ]

    def put_inputs_device(in_maps):
        import jax.numpy as jnp
        from jax.sharding import NamedSharding
        sh = NamedSharding(mesh, PartitionSpec("core"))
        return [jax.device_put(x, sh) for x in put_inputs(in_maps)]

    def make_zeros_device():
        import jax.numpy as jnp
        from jax.sharding import NamedSharding
        sh = NamedSharding(mesh, PartitionSpec("core"))
        fn = jax.jit(
            lambda: tuple(
                jnp.zeros((n_cores * a.shape[0], *a.shape[1:]), a.dtype)
                for a in out_avals),
            out_shardings=tuple(sh for _ in out_avals))
        return fn

    execute.put_inputs = put_inputs
    execute.put_inputs_device = put_inputs_device
    execute.make_zeros_device = make_zeros_device
    execute.raw = execute_raw
    execute.out_avals = out_avals
    execute.out_names = out_names
    execute.n_cores = n_cores
    return execute


def _get_runner():
    if "exec" not in _CACHE:
        nc = _build()
        _CACHE["exec"] = _make_exec(nc)
    return _CACHE["exec"]


def _shard(inputs):
    hs = np.asarray(inputs["hidden_states"], np.float32)
    ic = np.asarray(inputs["instruction_context"], np.float32)
    shared = {n: np.asarray(inputs[n], np.float32) for n in (
        "Wq", "Wk", "Wv", "Wa", "Wcs", "Wg",
        "bq", "bk", "bv", "ba", "bcs", "bg")}
    return [{"hs": hs[b], "ic": ic[b], **shared} for b in range(B)]


def kernel(**inputs):
    runner = _get_runner()
    results = runner(_shard(inputs))
    ctx = np.stack([r["ctx"] for r in results])             # [B, S, D]
    probsT = np.stack([r["probsT"] for r in results])       # [B, H, S_k, S_q]
    probs = probsT.transpose(0, 1, 3, 2)                    # [B, H, S_q, S_k]
    return ctx, probs


# revision 13
# speedup vs baseline: 3366.7307x; 46.0005x over previous
"""CodeAwareAttention Trainium2 kernel.

Shards batch (B=8) across 8 NeuronCores, one batch element per core.
Per core everything is computed in a "transposed" layout (feature dim on
partitions, tokens on the free axis) so that no large on-chip transposes of
the [S, S] attention matrices are needed:

  scoresT[k, q] = sum_d KT[d, k] * (Q[q, d]*gate'[q])  + cs'[q]   (aug row)
  probs_uT      = exp(scoresT) * exp(alignT)                       (softmax numerator)
  ctx_augT      = [V | 1]^T @ probs_uT          -> row 64 = softmax denominators
  probsT_out    = probs_uT * (1/den)[q]         (broadcast along partitions)
  ctx           = detranspose(ctx_augT) / den   (exact, per 128-token tile)

probs is written to HBM transposed ([H, S_k, S_q]); the host gather transposes
the view back to [B, H, S_q, S_k] (layout-only, no compute on host).

All matmuls run as float32r (full-rate fp32, ~2^-12 rounding on inputs).
"""

import numpy as np

import concourse.bass as bass
import concourse.tile as tile
from concourse import bacc, mybir
from concourse.masks import make_identity

F32 = mybir.dt.float32
F32R = mybir.dt.float32r
AF = mybir.ActivationFunctionType
ALU = mybir.AluOpType

B, S, D, H, DH = 8, 768, 768, 12, 64
NT = 6          # 128-row tiles per S/D
HALVES = ((0, 512), (512, 768))   # matmul moving-dim split; each fits one PSUM bank

_CACHE = {}


def _build():
    nc = bacc.Bacc("TRN2", target_bir_lowering=False, debug=False, num_devices=8)

    hs = nc.dram_tensor("hs", [S, D], F32, kind="ExternalInput").ap()
    ic = nc.dram_tensor("ic", [S, D], F32, kind="ExternalInput").ap()
    Wq = nc.dram_tensor("Wq", [D, D], F32, kind="ExternalInput").ap()
    Wk = nc.dram_tensor("Wk", [D, D], F32, kind="ExternalInput").ap()
    Wv = nc.dram_tensor("Wv", [D, D], F32, kind="ExternalInput").ap()
    Wa = nc.dram_tensor("Wa", [D, D], F32, kind="ExternalInput").ap()
    Wcs = nc.dram_tensor("Wcs", [D, H], F32, kind="ExternalInput").ap()
    Wg = nc.dram_tensor("Wg", [D, H], F32, kind="ExternalInput").ap()
    bq = nc.dram_tensor("bq", [D], F32, kind="ExternalInput").ap()
    bk = nc.dram_tensor("bk", [D], F32, kind="ExternalInput").ap()
    bv = nc.dram_tensor("bv", [D], F32, kind="ExternalInput").ap()
    ba = nc.dram_tensor("ba", [D], F32, kind="ExternalInput").ap()
    bcs = nc.dram_tensor("bcs", [H], F32, kind="ExternalInput").ap()
    bg = nc.dram_tensor("bg", [H], F32, kind="ExternalInput").ap()
    probsT = nc.dram_tensor("probsT", [H, S, S], F32, kind="ExternalOutput").ap()
    ctx_o = nc.dram_tensor("ctx", [S, D], F32, kind="ExternalOutput").ap()

    with tile.TileContext(nc) as tc:
        _emit(nc, tc, hs, ic, Wq, Wk, Wv, Wa, Wcs, Wg,
              bq, bk, bv, ba, bcs, bg, probsT, ctx_o)
    nc.compile()
    return nc


def _emit(nc, tc, hs, ic, Wq, Wk, Wv, Wa, Wcs, Wg,
          bq, bk, bv, ba, bcs, bg, probsT, ctx_o):
    from contextlib import ExitStack
    est = ExitStack()
    with est:
        cp = est.enter_context(tc.tile_pool(name="consts", bufs=1))
        pp = est.enter_context(tc.tile_pool(name="persist", bufs=1))

        ident = cp.tile([128, 128], F32, tag="ident")
        make_identity(nc, ident[:])
        ones_row = cp.tile([1, S], F32, tag="ones_row")
        nc.vector.memset(ones_row[:], 1.0)
        ones12 = cp.tile([128, H], F32, tag="ones12")
        nc.vector.memset(ones12[:], 1.0)

        bq_sb = cp.tile([128, NT], F32, tag="bq")
        nc.scalar.dma_start(bq_sb[:], bq.rearrange("(c p) -> p c", p=128))
        bk_sb = cp.tile([128, NT], F32, tag="bk")
        nc.scalar.dma_start(bk_sb[:], bk.rearrange("(c p) -> p c", p=128))
        ba_sb = cp.tile([128, NT], F32, tag="ba")
        nc.scalar.dma_start(ba_sb[:], ba.rearrange("(c p) -> p c", p=128))
        bcs_sb = cp.tile([H, 1], F32, tag="bcs")
        nc.scalar.dma_start(bcs_sb[:], bcs.rearrange("(h o) -> h o", o=1))
        bg_sb = cp.tile([H, 1], F32, tag="bg")
        nc.scalar.dma_start(bg_sb[:], bg.rearrange("(h o) -> h o", o=1))

        # persistent products of the preamble
        QA = [pp.tile([65, S], F32R, tag=f"QA{h}", name=f"QA{h}") for h in range(H)]
        KT = [pp.tile([65, S], F32R, tag=f"KT{h}", name=f"KT{h}") for h in range(H)]
        V = [pp.tile([128, H, 65], F32R, tag=f"V{t}", name=f"V{t}") for t in range(NT)]
        EA = [pp.tile([128, S], F32, tag=f"EA{j}", name=f"EA{j}") for j in range(NT)]

        with tc.tile_pool(name="phAB", bufs=1) as phAB:
            hsT = [phAB.tile([128, S], F32R, tag=f"hsT{j}", name=f"hsT{j}")
                   for j in range(NT)]
            icT = [phAB.tile([128, S], F32R, tag=f"icT{j}", name=f"icT{j}")
                   for j in range(NT)]

            # ---- Phase A: load hs/ic and transpose on the PE ----
            with tc.tile_pool(name="natp", bufs=2) as natp, \
                 tc.tile_pool(name="pstr", bufs=4, space="PSUM") as pstr:
                for src, dstT in ((hs, hsT), (ic, icT)):
                    for t2 in range(3):
                        nat = natp.tile([128, 2, S], F32, tag="nat")
                        nc.sync.dma_start(
                            nat[:], src[256 * t2:256 * t2 + 256, :]
                            .rearrange("(a p) d -> p a d", p=128))
                        for a in range(2):
                            i = 2 * t2 + a
                            for j in range(NT):
                                pt = pstr.tile([128, 128], F32, tag="tr")
                                nc.tensor.transpose(
                                    pt[:], nat[:, a, 128 * j:128 * j + 128],
                                    ident[:])
                                dst = dstT[j][:, 128 * i:128 * i + 128]
                                if j % 2 == 0:
                                    nc.scalar.copy(dst, pt[:])
                                else:
                                    nc.vector.tensor_copy(dst, pt[:])

            # ---- Phase B: projections ----
            with tc.tile_pool(name="wp", bufs=7) as wp, \
                 tc.tile_pool(name="phB", bufs=1) as phB, \
                 tc.tile_pool(name="stg", bufs=2) as stg, \
                 tc.tile_pool(name="tmpp", bufs=2) as tmpp, \
                 tc.tile_pool(name="gbp", bufs=2) as gbp, \
                 tc.tile_pool(name="psb", bufs=2, space="PSUM") as psb:

                def load_w(Wdram, label):
                    tiles = []
                    for c in range(NT):
                        t = wp.tile([128, S], F32R, tag="w", name=f"{label}{c}")
                        nc.sync.dma_start(
                            t[:], Wdram[128 * c:128 * c + 128, :].bitcast(F32R))
                        tiles.append(t)
                    return tiles

                # cs / gate projections ([12, S] each, partitions 0-11)
                wcsg = phB.tile([128, NT, 2 * H], F32R, tag="wcsg")
                nc.scalar.dma_start(
                    wcsg[:, :, 0:H],
                    Wcs.rearrange("(c p) h -> p c h", p=128).bitcast(F32R))
                nc.scalar.dma_start(
                    wcsg[:, :, H:2 * H],
                    Wg.rearrange("(c p) h -> p c h", p=128).bitcast(F32R))
                ps_cs = psb.tile([H, S], F32, tag="csg")
                ps_g = psb.tile([H, S], F32, tag="csg")
                for c in range(NT):
                    for lo, hi in HALVES:
                        nc.tensor.matmul(ps_cs[:, lo:hi], wcsg[:, c, 0:H],
                                         hsT[c][:, lo:hi],
                                         start=(c == 0), stop=(c == NT - 1))
                        nc.tensor.matmul(ps_g[:, lo:hi], wcsg[:, c, H:2 * H],
                                         hsT[c][:, lo:hi],
                                         start=(c == 0), stop=(c == NT - 1))
                cs_sb = phB.tile([H, S], F32, tag="cs")
                nc.scalar.add(cs_sb[:], ps_cs[:], bcs_sb[:])
                gate_sb = phB.tile([H, S], F32, tag="gate")
                nc.scalar.activation(gate_sb[:], ps_g[:], AF.Sigmoid,
                                     bias=bg_sb[:], scale=1.0)
                cprime = phB.tile([H, S], F32, tag="cp")
                nc.vector.tensor_mul(cprime[:], cs_sb[:], gate_sb[:])
                gate8 = phB.tile([H, S], F32, tag="g8")
                nc.vector.tensor_scalar_mul(gate8[:], gate_sb[:], 0.125)

                # Q projection, fused with gate prescale -> QA head tiles
                wq_sb = load_w(Wq, "wq")
                for m in range(NT):
                    pj = psb.tile([128, S], F32, tag="proj")
                    for c in range(NT):
                        for lo, hi in HALVES:
                            nc.tensor.matmul(pj[:, lo:hi],
                                             wq_sb[c][:, 128 * m:128 * m + 128],
                                             hsT[c][:, lo:hi],
                                             start=(c == 0), stop=(c == NT - 1))
                    he, ho = 2 * m, 2 * m + 1
                    stge = stg.tile([1, S], F32, tag="stg")
                    nc.scalar.dma_start(stge[:], gate8[he:he + 1, :])
                    gbe = gbp.tile([128, S], F32, tag="gb")
                    nc.gpsimd.partition_broadcast(gbe[:], stge[:])
                    stgo = stg.tile([1, S], F32, tag="stg")
                    nc.scalar.dma_start(stgo[:], gate8[ho:ho + 1, :])
                    gbo = gbp.tile([128, S], F32, tag="gb")
                    nc.gpsimd.partition_broadcast(gbo[:], stgo[:])
                    nc.vector.scalar_tensor_tensor(
                        QA[he][0:64, :], pj[0:64, :], bq_sb[0:64, m:m + 1],
                        gbe[0:64, :], op0=ALU.add, op1=ALU.mult)
                    qtmp = tmpp.tile([128, S], F32R, tag="qtmp")
                    nc.vector.scalar_tensor_tensor(
                        qtmp[64:128, :], pj[64:128, :], bq_sb[64:128, m:m + 1],
                        gbo[64:128, :], op0=ALU.add, op1=ALU.mult)
                    nc.scalar.dma_start(QA[ho][0:64, :], qtmp[64:128, :])
                    nc.scalar.dma_start(QA[he][64:65, :],
                                        cprime[he:he + 1, :].bitcast(F32R))
                    nc.scalar.dma_start(QA[ho][64:65, :],
                                        cprime[ho:ho + 1, :].bitcast(F32R))

                # K projection -> KT head tiles (row 64 = ones)
                wk_sb = load_w(Wk, "wk")
                for m in range(NT):
                    pj = psb.tile([128, S], F32, tag="proj")
                    for c in range(NT):
                        for lo, hi in HALVES:
                            nc.tensor.matmul(pj[:, lo:hi],
                                             wk_sb[c][:, 128 * m:128 * m + 128],
                                             hsT[c][:, lo:hi],
                                             start=(c == 0), stop=(c == NT - 1))
                    he, ho = 2 * m, 2 * m + 1
                    nc.scalar.add(KT[he][0:64, :], pj[0:64, :], bk_sb[0:64, m:m + 1])
                    ktmp = tmpp.tile([128, S], F32R, tag="qtmp")
                    nc.scalar.add(ktmp[64:128, :], pj[64:128, :],
                                  bk_sb[64:128, m:m + 1])
                    nc.scalar.dma_start(KT[ho][0:64, :], ktmp[64:128, :])
                    nc.scalar.dma_start(KT[he][64:65, :], ones_row[:].bitcast(F32R))
                    nc.scalar.dma_start(KT[ho][64:65, :], ones_row[:].bitcast(F32R))

                # V projection (natural layout, per-head 65-wide with ones col)
                wv_sb = load_w(Wv, "wv")
                stbv = stg.tile([1, S], F32, tag="stg")
                nc.scalar.dma_start(stbv[:], bv.rearrange("(o d) -> o d", o=1))
                bv_b = phB.tile([128, S], F32, tag="bvb")
                nc.gpsimd.partition_broadcast(bv_b[:], stbv[:])
                for t in range(NT):
                    pj = psb.tile([128, S], F32, tag="proj")
                    for c in range(NT):
                        for lo, hi in HALVES:
                            nc.tensor.matmul(pj[:, lo:hi],
                                             hsT[c][:, 128 * t:128 * t + 128],
                                             wv_sb[c][:, lo:hi],
                                             start=(c == 0), stop=(c == NT - 1))
                    nc.vector.tensor_add(
                        V[t][:, :, 0:64],
                        pj[:].rearrange("p (h d) -> p h d", h=H),
                        bv_b[:].rearrange("p (h d) -> p h d", h=H))
                    nc.scalar.dma_start(
                        V[t][:, :, 64:65],
                        ones12[:].rearrange("p (h o) -> p h o", o=1).bitcast(F32R))

                # align projection -> exp(alignT + ba) directly
                wa_sb = load_w(Wa, "wa")
                for kt in range(NT):
                    pj = psb.tile([128, S], F32, tag="proj")
                    for c in range(NT):
                        for lo, hi in HALVES:
                            nc.tensor.matmul(pj[:, lo:hi],
                                             wa_sb[c][:, 128 * kt:128 * kt + 128],
                                             icT[c][:, lo:hi],
                                             start=(c == 0), stop=(c == NT - 1))
                    nc.scalar.activation(EA[kt][:], pj[:], AF.Exp,
                                         bias=ba_sb[:, kt:kt + 1], scale=1.0)

        # ---- Phase C: attention per head ----
        with tc.tile_pool(name="ctxp", bufs=1) as ctxp, \
             tc.tile_pool(name="e1p", bufs=3) as e1p, \
             tc.tile_pool(name="pup", bufs=8) as pup, \
             tc.tile_pool(name="pop", bufs=2) as pop, \
             tc.tile_pool(name="catp", bufs=2) as catp, \
             tc.tile_pool(name="invbp", bufs=2) as invbp, \
             tc.tile_pool(name="stgC", bufs=2) as stgC, \
             tc.tile_pool(name="psS", bufs=2, space="PSUM") as psS, \
             tc.tile_pool(name="psC", bufs=1, space="PSUM") as psC, \
             tc.tile_pool(name="psT", bufs=2, space="PSUM") as psT:

            ctx_sb = [ctxp.tile([128, 2, S], F32, tag=f"ctx{t2}", name=f"ctxsb{t2}")
                      for t2 in range(3)]

            for h in range(H):
                pu_tiles = []
                for kt in range(NT):
                    ps_s = psS.tile([128, S], F32, tag="sc")
                    for lo, hi in HALVES:
                        nc.tensor.matmul(ps_s[:, lo:hi],
                                         KT[h][:, 128 * kt:128 * kt + 128],
                                         QA[h][:, lo:hi], start=True, stop=True)
                    e1 = e1p.tile([128, S], F32, tag="e1")
                    nc.scalar.activation(e1[:], ps_s[:], AF.Exp)
                    pu = pup.tile([128, S], F32R, tag="pu")
                    eng = nc.vector if kt % 2 == 0 else nc.gpsimd
                    eng.tensor_mul(pu[:], e1[:], EA[kt][:])
                    pu_tiles.append(pu)

                ps_c = psC.tile([65, S], F32, tag="cx")
                for kt in range(NT):
                    for lo, hi in HALVES:
                        nc.tensor.matmul(ps_c[:, lo:hi], V[kt][:, h, :],
                                         pu_tiles[kt][:, lo:hi],
                                         start=(kt == 0), stop=(kt == NT - 1))
                cat = catp.tile([65, S], F32, tag="cat")
                nc.scalar.copy(cat[:], ps_c[:])
                den = stgC.tile([1, S], F32, tag="den")
                nc.scalar.dma_start(den[:], cat[64:65, :])
                inv = stgC.tile([1, S], F32, tag="inv")
                nc.vector.reciprocal_approx_fast(out=inv[:], in_=den[:])
                invb = invbp.tile([128, S], F32, tag="invb")
                nc.gpsimd.partition_broadcast(invb[:], inv[:])

                for g in range(2):
                    po = pop.tile([128, 3, S], F32, tag="po")
                    for j in range(3):
                        kt = 3 * g + j
                        eng = nc.gpsimd if kt % 2 == 0 else nc.vector
                        eng.tensor_mul(po[:, j, :],
                                       pu_tiles[kt][:].bitcast(F32), invb[:])
                    nc.sync.dma_start(
                        probsT[h, 384 * g:384 * g + 384, :]
                        .rearrange("(j p) d -> p j d", p=128),
                        po[:])

                # ctx: detranspose ctx_augT, divide by den, accumulate to ctx_sb
                pt6 = psT.tile([128, NT, 80], F32, tag="dt")
                for qt in range(NT):
                    nc.tensor.transpose(pt6[:, qt, 0:65],
                                        cat[:, 128 * qt:128 * qt + 128],
                                        ident[0:65, 0:65])
                inv6 = stgC.tile([128, NT], F32, tag="i6")
                nc.vector.reciprocal(
                    inv6[:], pt6[:, :, 64:65].rearrange("p c o -> p (c o)"))
                for qt in range(NT):
                    nc.vector.tensor_scalar_mul(
                        ctx_sb[qt // 2][:, qt % 2, 64 * h:64 * h + 64],
                        pt6[:, qt, 0:64], inv6[:, qt:qt + 1])

            for t2 in range(3):
                nc.sync.dma_start(
                    ctx_o[256 * t2:256 * t2 + 256, :]
                    .rearrange("(a p) d -> p a d", p=128),
                    ctx_sb[t2][:])


def _make_exec(nc):
    """Build a cached jitted SPMD executor (mirrors bass2jax.run_bass_via_pjrt)."""
    import jax
    from jax.sharding import Mesh, PartitionSpec
    from jax.experimental.shard_map import shard_map
    from concourse import bass2jax as b2j
    from concourse import mybir as _mybir

    b2j.install_neuronx_cc_hook()
    partition_name = (nc.partition_id_tensor.name
                      if nc.partition_id_tensor else None)

    assert nc.dbg_addr is None, "build with debug=False"
    in_names, out_names, out_avals = [], [], []
    for alloc in nc.m.functions[0].allocations:
        if not isinstance(alloc, _mybir.MemoryLocationSet):
            continue
        name = alloc.memorylocations[0].name
        if alloc.kind == "ExternalInput":
            if name != partition_name:
                in_names.append(name)
        elif alloc.kind == "ExternalOutput":
            out_names.append(name)
            out_avals.append(jax.core.ShapedArray(
                tuple(alloc.tensor_shape), _mybir.dt.np(alloc.dtype)))

    n_params = len(in_names)
    n_outs = len(out_names)
    all_in_names = list(in_names) + list(out_names)
    if partition_name is not None:
        all_in_names.append(partition_name)
    donate = tuple(range(n_params, n_params + n_outs))

    def _body(*args):
        operands = list(args)
        if partition_name is not None:
            operands.append(b2j.partition_id_tensor())
        outs = b2j._bass_exec_p.bind(
            *operands,
            out_avals=tuple(jax.core.ShapedArray(a.shape, a.dtype)
                            for a in out_avals),
            in_names=tuple(all_in_names),
            out_names=tuple(out_names),
            lowering_input_output_aliases=(),
            sim_require_finite=True,
            sim_require_nnan=True,
            nc=nc,
        )
        return tuple(outs)

    n_cores = 8
    devices = jax.devices()[:n_cores]
    mesh = Mesh(np.asarray(devices), ("core",))
    in_specs = (PartitionSpec("core"),) * (n_params + n_outs)
    out_specs = (PartitionSpec("core"),) * n_outs
    sharded = jax.jit(
        shard_map(_body, mesh=mesh, in_specs=in_specs, out_specs=out_specs,
                  check_rep=False),
        donate_argnums=donate, keep_unused=True)

    def put_inputs(in_maps):
        return [np.concatenate([np.asarray(in_maps[c][nm])[None]
                                if np.asarray(in_maps[c][nm]).ndim == 0
                                else np.asarray(in_maps[c][nm])
                                for c in range(n_cores)], axis=0)
                for nm in in_names]

    def execute_raw(device_inputs, zeros):
        return sharded(*device_inputs, *zeros)

    def execute(in_maps):
        device_inputs = put_inputs(in_maps)
        zeros = [np.zeros((n_cores * a.shape[0], *a.shape[1:]), a.dtype)
                 for a in out_avals]
        out_arrs = sharded(*device_inputs, *zeros)
        return [
            {name: np.asarray(out_arrs[i]).reshape(n_cores, *out_avals[i].shape)[c]
             for i, name in enumerate(out_names)}
            for c in range(n_cores)
        ]

    def make_loop_exec(n_iters):
        """Jitted callable executing the NEFF n_iters times back-to-back on
        device (single dispatch) — for slope-based kernel timing. Iterations
        are chained via a zero-scaled scalar folded into the first input so
        they cannot be deduplicated/overlapped."""
        import jax.numpy as jnp

        def _body_n(*args):
            outs = None
            args = list(args)
            for _ in range(n_iters):
                operands = list(args)
                if partition_name is not None:
                    operands.append(b2j.partition_id_tensor())
                outs = b2j._bass_exec_p.bind(
                    *operands,
                    out_avals=tuple(jax.core.ShapedArray(a.shape, a.dtype)
                                    for a in out_avals),
                    in_names=tuple(all_in_names),
                    out_names=tuple(out_names),
                    lowering_input_output_aliases=(),
                    sim_require_finite=True,
                    sim_require_nnan=True,
                    nc=nc,
                )
                # data-dependency chain: next iteration's first input
                # depends on this iteration's first output (scaled by 0)
                args[0] = args[0] + outs[0].ravel()[0] * 0.0
            return tuple(outs)

        return jax.jit(
            shard_map(_body_n, mesh=mesh, in_specs=in_specs,
                      out_specs=out_specs, check_rep=False),
            keep_unused=True)

    sharded_nodonate = jax.jit(
        shard_map(_body, mesh=mesh, in_specs=in_specs, out_specs=out_specs,
                  check_rep=False),
        keep_unused=True)

    def raw_nodonate(device_inputs, zeros):
        return sharded_nodonate(*device_inputs, *zeros)

    def put_inputs_device(in_maps):
        import jax.numpy as jnp
        from jax.sharding import NamedSharding
        sh = NamedSharding(mesh, PartitionSpec("core"))
        return [jax.device_put(x, sh) for x in put_inputs(in_maps)]

    def make_zeros_device():
        import jax.numpy as jnp
        from jax.sharding import NamedSharding
        sh = NamedSharding(mesh, PartitionSpec("core"))
        fn = jax.jit(
            lambda: tuple(
                jnp.zeros((n_cores * a.shape[0], *a.shape[1:]), a.dtype)
                for a in out_avals),
            out_shardings=tuple(sh for _ in out_avals))
        return fn

    execute.put_inputs = put_inputs
    execute.put_inputs_device = put_inputs_device
    execute.make_zeros_device = make_zeros_device
    execute.make_loop_exec = make_loop_exec
    execute.raw = execute_raw
    execute.raw_nodonate = raw_nodonate
    execute.out_avals = out_avals
    execute.out_names = out_names
    execute.n_cores = n_cores
    return execute


def _get_runner():
    if "exec" not in _CACHE:
        nc = _build()
        _CACHE["exec"] = _make_exec(nc)
    return _CACHE["exec"]


def _shard(inputs):
    hs = np.asarray(inputs["hidden_states"], np.float32)
    ic = np.asarray(inputs["instruction_context"], np.float32)
    shared = {n: np.asarray(inputs[n], np.float32) for n in (
        "Wq", "Wk", "Wv", "Wa", "Wcs", "Wg",
        "bq", "bk", "bv", "ba", "bcs", "bg")}
    return [{"hs": hs[b], "ic": ic[b], **shared} for b in range(B)]


def kernel(**inputs):
    runner = _get_runner()
    results = runner(_shard(inputs))
    ctx = np.stack([r["ctx"] for r in results])             # [B, S, D]
    probsT = np.stack([r["probsT"] for r in results])       # [B, H, S_k, S_q]
    probs = probsT.transpose(0, 1, 3, 2)                    # [B, H, S_q, S_k]
    return ctx, probs
